# revision 1
# baseline (speedup 1.0000x reference)
"""Chamfer distance loss kernel for Trainium2 (Bass/Tile), 8-core SPMD.

Problem: B=8 batches of N=8192 source / M=8192 target 3-D points.
  dist[n,m] = |s_n|^2 + |t_m|^2 - 2 s.t
  chamfer[b] = mean_n min_m dist + mean_m min_n dist

Sharding: data-parallel over batch; core b handles batch b end-to-end and
emits one scalar. No cross-core communication.

Per-core pipeline:
  PE  : K=5 augmented matmul [s,1,|s|^2].[-2t,|t|^2,1] -> PSUM fp32 = dist
  ACT : PSUM -> SBUF bf16 cast (plain Copy)
  DVE : bf16 2x tensor_tensor min-accumulate (col) + binary fold tree (row)
  PE  : transpose col accumulator for the cross-partition min; ones-matmul
        for the final partition sum
"""

import ml_dtypes
import numpy as np

import concourse.bacc as bacc
import concourse.bass as bass
import concourse.mybir as mybir
import concourse.tile as tile
from concourse.bass_utils import run_bass_kernel_spmd

B = 8
N = 8192  # source points per batch
M = 8192  # target points per batch
D = 3

NT = N // 128  # 64 source tiles of 128
QCH = 2048     # ACT/DVE chunk width (4 PSUM banks)
NH = M // QCH  # 4 chunks per source tile row
BIG = 60000.0  # > any squared distance here, fp16-safe

F32 = mybir.dt.float32
F16 = mybir.dt.bfloat16
MIN = mybir.AluOpType.min
ADD = mybir.AluOpType.add


def _build_kernel(nc: bass.Bass, src_d, tgt_d, out_d, reps=1):
    tc_ctx = tile.TileContext(nc)
    with tc_ctx as tc, tc.tile_pool(name="const", bufs=1) as cpool:
        with tc.tile_pool(name="prep", bufs=1) as prep:
            # Persistent SBUF tensors
            aug_s = cpool.tile([5, N], F32)       # rows: s_x, s_y, s_z, 1, |s|^2
            aug_t = cpool.tile([5, M], F32)       # rows: -2t_x, -2t_y, -2t_z, |t|^2, 1
            col_acc = cpool.tile([128, M], F16)   # min over n of dist, [p, m]
            rowmins = cpool.tile([128, NT], F32)  # min over m of dist, [p, c]
            colmins = cpool.tile([128, NT], F32)  # per-128-m-chunk col mins
            ident = cpool.tile([128, 128], F16)   # identity for PE transpose
            ones128 = cpool.tile([128, 1], F32)   # final partition-sum weights

            id_dram = nc.inline_tensor(np.eye(128, dtype=np.float32).astype(ml_dtypes.bfloat16), name="ident")
            nc.sync.dma_start(ident[:], id_dram.ap())
            nc.gpsimd.memset(ones128[:], 1.0)
            ones_dram = nc.inline_tensor(np.ones((1, N), dtype=np.float32), name="ones_row")

            # ---- input prep ----
            # coord rows via strided DMA [d, n]
            nc.sync.dma_start(aug_s[0:3, :], src_d.ap().rearrange("n d -> d n"))
            nc.sync.dma_start(aug_t[0:3, :], tgt_d.ap().rearrange("m d -> d m"))
            nc.sync.dma_start(aug_s[3:4, :], ones_dram.ap())
            nc.sync.dma_start(aug_t[4:5, :], ones_dram.ap())
            # scale target rows by -2 (in place)
            nc.vector.tensor_scalar_mul(aug_t[0:3, :], aug_t[0:3, :], -2.0)

            # |t|^2 row: square scaled rows, ones-matmul with 0.25 weights
            sq_t = prep.tile([3, M], F32, tag="sq")
            nc.scalar.square(sq_t[:], aug_t[0:3, :])
            w025 = prep.tile([3, 1], F32)
            nc.gpsimd.memset(w025[:], 0.25)
            tsq_tmp = prep.tile([1, M], F32, tag="tmp")
            with tc.tile_pool(name="psum_prep", bufs=2, space=bass.MemorySpace.PSUM) as pprep:
                for quarter in range(4):
                    pt = pprep.tile([1, 2048], F32)
                    for q in range(4):
                        mq = quarter * 2048 + q * 512
                        nc.tensor.matmul(
                            pt[:, q * 512:(q + 1) * 512],
                            w025[:],
                            sq_t[:, mq:mq + 512],
                        )
                    nc.scalar.copy(tsq_tmp[:, quarter * 2048:(quarter + 1) * 2048], pt[:])
            nc.sync.dma_start(aug_t[3:4, :], tsq_tmp[:])

            # |s|^2 row via square + ones-matmul (weights 1.0)
            sq_s = prep.tile([3, N], F32, tag="sq")
            nc.scalar.square(sq_s[:], aug_s[0:3, :])
            w1 = prep.tile([3, 1], F32)
            nc.gpsimd.memset(w1[:], 1.0)
            ssq_tmp = prep.tile([1, N], F32, tag="tmp")
            with tc.tile_pool(name="psum_prep2", bufs=2, space=bass.MemorySpace.PSUM) as pprep2:
                for quarter in range(4):
                    pt2 = pprep2.tile([1, 2048], F32)
                    for q in range(4):
                        nq = quarter * 2048 + q * 512
                        nc.tensor.matmul(
                            pt2[:, q * 512:(q + 1) * 512],
                            w1[:],
                            sq_s[:, nq:nq + 512],
                        )
                    nc.scalar.copy(ssq_tmp[:, quarter * 2048:(quarter + 1) * 2048], pt2[:])
            nc.sync.dma_start(aug_s[4:5, :], ssq_tmp[:])

        # ---- main loop (reps>1 only for exec-time measurement) ----
        for _rep in range(reps):
          with (
            tc.tile_pool(name="dpsum", bufs=2, space=bass.MemorySpace.PSUM) as dpsum,
            tc.tile_pool(name="d16", bufs=2) as d16p,
            tc.tile_pool(name="rowacc", bufs=1) as rowp,
          ):
            for c in range(NT):
                lhsT = aug_s[:, c * 128:(c + 1) * 128]
                d16 = d16p.tile([128, M], F16)
                for h in range(NH):
                    dps = dpsum.tile([128, QCH], F32)
                    for q in range(QCH // 512):
                        mq = h * QCH + q * 512
                        nc.tensor.matmul(
                            dps[:, q * 512:(q + 1) * 512],
                            lhsT,
                            aug_t[:, mq:mq + 512],
                        )
                    # fp32 PSUM -> fp16 SBUF slice of the full row block
                    nc.scalar.copy(d16[:, h * QCH:(h + 1) * QCH], dps[:])
                # column (min over n) accumulate: one wide op
                if c == 0:
                    nc.vector.tensor_copy(col_acc[:], d16[:])
                else:
                    nc.vector.tensor_tensor(col_acc[:], d16[:], col_acc[:], op=MIN)
                # row (min over m): binary fold tree then one short 1x reduce
                rowh = rowp.tile([128, M // 2], F16)
                nc.vector.tensor_tensor(
                    rowh[:], d16[:, 0:M // 2], d16[:, M // 2:M], op=MIN
                )
                for w in (M // 4, M // 8, M // 16):
                    nc.vector.tensor_tensor(
                        rowh[:, 0:w], rowh[:, 0:w], rowh[:, w:2 * w], op=MIN
                    )
                nc.vector.tensor_reduce(
                    rowmins[:, c:c + 1], rowh[:, 0:M // 16],
                    axis=mybir.AxisListType.X, op=MIN,
                )

        # ---- column partition-reduce via PE transpose ----
        with tc.tile_pool(name="tpsum", bufs=4, space=bass.MemorySpace.PSUM) as tpsum:
            for c in range(NT):
                tps = tpsum.tile([128, 128], F16)
                nc.tensor.transpose(tps[:], col_acc[:, c * 128:(c + 1) * 128], ident[:])
                nc.vector.tensor_reduce(
                    colmins[:, c:c + 1], tps[:], axis=mybir.AxisListType.X, op=MIN
                )

        # ---- final scalar ----
        with (
            tc.tile_pool(name="fin", bufs=1) as fin,
            tc.tile_pool(name="fpsum", bufs=1, space=bass.MemorySpace.PSUM) as fpsum,
        ):
            sums = fin.tile([128, 2], F32)
            nc.vector.tensor_reduce(
                sums[:, 0:1], rowmins[:], axis=mybir.AxisListType.X, op=ADD
            )
            nc.vector.tensor_reduce(
                sums[:, 1:2], colmins[:], axis=mybir.AxisListType.X, op=ADD
            )
            tot = fin.tile([128, 1], F32)
            nc.vector.tensor_tensor(tot[:], sums[:, 0:1], sums[:, 1:2], op=ADD)
            ps = fpsum.tile([1, 1], F32)
            nc.tensor.matmul(ps[:], tot[:], ones128[:])
            res = fin.tile([1, 1], F32)
            nc.scalar.mul(res[:], ps[:], 1.0 / float(N))
            nc.sync.dma_start(out_d.ap(), res[:])


_NC_CACHE = {}


def _get_nc(reps=1):
    if reps not in _NC_CACHE:
        nc = bacc.Bacc("TRN2", target_bir_lowering=False, debug=False)
        src_d = nc.dram_tensor("src", [N, D], F32, kind="ExternalInput")
        tgt_d = nc.dram_tensor("tgt", [M, D], F32, kind="ExternalInput")
        out_d = nc.dram_tensor("out", [1, 1], F32, kind="ExternalOutput")
        _build_kernel(nc, src_d, tgt_d, out_d, reps=reps)
        nc.compile()
        _NC_CACHE[reps] = nc
    return _NC_CACHE[reps]


def kernel(source_points: np.ndarray, target_points: np.ndarray) -> np.ndarray:
    src = np.ascontiguousarray(np.asarray(source_points), dtype=np.float32)
    tgt = np.ascontiguousarray(np.asarray(target_points), dtype=np.float32)
    assert src.shape == (B, N, D) and tgt.shape == (B, M, D)

    nc = _get_nc()
    in_maps = [{"src": src[b], "tgt": tgt[b]} for b in range(B)]
    res = run_bass_kernel_spmd(nc, in_maps, list(range(B)))
    return np.stack(
        [res.results[b]["out"].reshape(()) for b in range(B)]
    ).astype(np.float32)


if __name__ == "__main__":
    rng = np.random.default_rng(0)
    s = rng.standard_normal((B, N, D), dtype=np.float32)
    t = rng.standard_normal((B, M, D), dtype=np.float32)
    print(kernel(s, t))



# revision 7
# speedup vs baseline: 3.0709x; 3.0709x over previous
"""Chamfer distance loss kernel for Trainium2 (Bass/Tile), 8-core SPMD.

Problem: B=8 batches of N=8192 source / M=8192 target 3-D points.
  dist[n,m] = |s_n|^2 + |t_m|^2 - 2 s.t
  chamfer[b] = mean_n min_m dist + mean_m min_n dist

Sharding: data-parallel over batch; core b handles batch b end-to-end and
emits one scalar. No cross-core communication.

Per-core pipeline:
  PE  : K=5 augmented matmul [s,1,|s|^2].[-2t,|t|^2,1] -> PSUM fp32 = dist
  ACT : PSUM -> SBUF bf16 cast (plain Copy)
  DVE : bf16 2x tensor_tensor min-accumulate (col) + binary fold tree (row)
  PE  : transpose col accumulator for the cross-partition min; ones-matmul
        for the final partition sum
"""

import ml_dtypes
import numpy as np

import concourse.bacc as bacc
import concourse.bass as bass
import concourse.mybir as mybir
import concourse.tile as tile
from concourse.bass_utils import run_bass_kernel_spmd

B = 8
N = 8192  # source points per batch
M = 8192  # target points per batch
D = 3

NT = N // 128  # 64 source tiles of 128
QCH = 2048     # ACT/DVE chunk width (4 PSUM banks)
NH = M // QCH  # 4 chunks per source tile row
BIG = 60000.0  # > any squared distance here, fp16-safe

F32 = mybir.dt.float32
F32R = mybir.dt.float32r  # fp32 bits, full-rate PE streaming (1 cyc/col at N>=256)
F16 = mybir.dt.bfloat16
MIN = mybir.AluOpType.min
ADD = mybir.AluOpType.add


def _build_kernel(nc: bass.Bass, src_d, tgt_d, out_d, reps=1):
    tc_ctx = tile.TileContext(nc)
    with tc_ctx as tc, tc.tile_pool(name="const", bufs=1) as cpool:
        with tc.tile_pool(name="prep", bufs=1) as prep:
            # Persistent SBUF tensors.
            # K=7 augmented operands (fp32r): the cross term rides rows 0-2
            # with coords pre-rounded to fp32r (exact products of perturbed
            # points), and each squared-norm rides as a hi/lo fp32r pair so
            # PSUM receives the full distance at ~fp32 precision while the
            # PE streams at full rate (1 cyc/col).
            #   aug_s rows: s_x, s_y, s_z, 1, 1, |s|^2_hi, |s|^2_lo
            #   aug_t rows: -2t_x, -2t_y, -2t_z, |t|^2_hi, |t|^2_lo, 1, 1
            aug_s_r = cpool.tile([7, N], F32R)
            aug_t_r = cpool.tile([7, M], F32R)
            col_acc = cpool.tile([128, M], F16)   # min over n of dist, [p, m]
            rowmins = cpool.tile([128, NT], F32)  # min over m of dist, [p, c]
            colmins = cpool.tile([128, NT], F32)  # per-128-m-chunk col mins
            ident = cpool.tile([128, 128], F16)   # identity for PE transpose
            ones128 = cpool.tile([128, 1], F32)   # final partition-sum weights

            id_dram = nc.inline_tensor(np.eye(128, dtype=np.float32).astype(ml_dtypes.bfloat16), name="ident")
            nc.sync.dma_start(ident[:], id_dram.ap())
            nc.gpsimd.memset(ones128[:], 1.0)

            # ---- input prep ----
            # DVE/ACT ops can only address partition bases {0,32,64,96}, so
            # each aug row group is produced (with fp32r rounding) in a
            # partition-0-based staging tile and DMA'd into place; DMA from
            # an fp32r source keeps the rounded provenance the fp32r matmul
            # verifier demands.  Norms are computed FROM the rounded coords,
            # so PSUM receives the exact squared distance of the perturbed
            # point set (plus the tiny hi/lo residual).
            stage = prep.tile([3, M], F32, tag="stage")   # raw coords / scratch
            crd_r = prep.tile([3, M], F32R, tag="crdr")   # rounded coords / scratch
            sq = prep.tile([3, M], F32, tag="sq")
            w_t = prep.tile([3, 1], F32)
            w_s = prep.tile([3, 1], F32)
            nc.gpsimd.memset(w_t[:], 0.25)
            nc.gpsimd.memset(w_s[:], 1.0)
            SUB = mybir.AluOpType.subtract

            def _prep_side(src_dram, n_elems, aug, coord_scale, w, hi_row,
                           ones_row, tag):
                nc.sync.dma_start(
                    stage[:, 0:n_elems], src_dram.ap().rearrange("n d -> d n")
                )
                # rounded (scaled) coords -> aug rows 0-2
                if coord_scale == 1.0:
                    nc.vector.tensor_copy(crd_r[:, 0:n_elems], stage[:, 0:n_elems])
                else:
                    nc.vector.tensor_scalar_mul(
                        crd_r[:, 0:n_elems], stage[:, 0:n_elems], coord_scale
                    )
                nc.sync.dma_start(aug[0:3, :], crd_r[:, 0:n_elems])
                # norm^2 = w * sum of squares of the (scaled) rounded coords
                nc.scalar.square(sq[:, 0:n_elems], crd_r[:, 0:n_elems].bitcast(F32))
                nsq = stage[0:1]  # raw coords dead once crd_r is built
                with tc.tile_pool(
                    name="psum_prep" + tag, bufs=2, space=bass.MemorySpace.PSUM
                ) as pprep:
                    for quarter in range(n_elems // 2048):
                        pt = pprep.tile([1, 2048], F32)
                        for q in range(4):
                            mq = quarter * 2048 + q * 512
                            nc.tensor.matmul(
                                pt[:, q * 512:(q + 1) * 512],
                                w[:],
                                sq[:, mq:mq + 512],
                            )
                        nc.scalar.copy(
                            nsq[:, quarter * 2048:(quarter + 1) * 2048], pt[:]
                        )
                # hi/lo split on the fp32r lattice, staged through crd_r[0:1]
                nc.vector.tensor_copy(crd_r[0:1, 0:n_elems], nsq[:, 0:n_elems])
                nc.sync.dma_start(aug[hi_row:hi_row + 1, :], crd_r[0:1, 0:n_elems])
                nc.vector.tensor_tensor(
                    crd_r[0:1, 0:n_elems], nsq[:, 0:n_elems],
                    crd_r[0:1, 0:n_elems].bitcast(F32), op=SUB,
                )
                nc.sync.dma_start(
                    aug[hi_row + 1:hi_row + 2, :], crd_r[0:1, 0:n_elems]
                )
                # ones rows
                nc.gpsimd.memset(stage[0:2, 0:n_elems], 1.0)
                nc.vector.tensor_copy(crd_r[0:2, 0:n_elems], stage[0:2, 0:n_elems])
                nc.sync.dma_start(
                    aug[ones_row:ones_row + 2, :], crd_r[0:2, 0:n_elems]
                )

            _prep_side(tgt_d, M, aug_t_r, -2.0, w_t, 3, 5, "t")
            _prep_side(src_d, N, aug_s_r, 1.0, w_s, 5, 3, "s")

        # ---- main loop (reps>1 only for exec-time measurement) ----
        for _rep in range(reps):
          with (
            tc.tile_pool(name="dpsum", bufs=2, space=bass.MemorySpace.PSUM) as dpsum,
            tc.tile_pool(name="d16", bufs=2) as d16p,
            tc.tile_pool(name="rowacc", bufs=1) as rowp,
          ):
            for c in range(NT):
                lhsT = aug_s_r[:, c * 128:(c + 1) * 128]
                d16 = d16p.tile([128, M], F16)
                for h in range(NH):
                    dps = dpsum.tile([128, QCH], F32)
                    for q in range(QCH // 512):
                        mq = h * QCH + q * 512
                        nc.tensor.matmul(
                            dps[:, q * 512:(q + 1) * 512],
                            lhsT,
                            aug_t_r[:, mq:mq + 512],
                        )
                    # fp32 PSUM -> fp16 SBUF slice of the full row block
                    nc.scalar.copy(d16[:, h * QCH:(h + 1) * QCH], dps[:])
                # column (min over n) accumulate: one wide op
                if c == 0:
                    nc.vector.tensor_copy(col_acc[:], d16[:])
                else:
                    nc.vector.tensor_tensor(col_acc[:], d16[:], col_acc[:], op=MIN)
                # row (min over m): binary fold tree then one short 1x reduce
                rowh = rowp.tile([128, M // 2], F16)
                nc.vector.tensor_tensor(
                    rowh[:], d16[:, 0:M // 2], d16[:, M // 2:M], op=MIN
                )
                for w in (M // 4, M // 8, M // 16):
                    nc.vector.tensor_tensor(
                        rowh[:, 0:w], rowh[:, 0:w], rowh[:, w:2 * w], op=MIN
                    )
                nc.vector.tensor_reduce(
                    rowmins[:, c:c + 1], rowh[:, 0:M // 16],
                    axis=mybir.AxisListType.X, op=MIN,
                )

        # ---- column partition-reduce via PE transpose ----
        with tc.tile_pool(name="tpsum", bufs=4, space=bass.MemorySpace.PSUM) as tpsum:
            for c in range(NT):
                tps = tpsum.tile([128, 128], F16)
                nc.tensor.transpose(tps[:], col_acc[:, c * 128:(c + 1) * 128], ident[:])
                nc.vector.tensor_reduce(
                    colmins[:, c:c + 1], tps[:], axis=mybir.AxisListType.X, op=MIN
                )

        # ---- final scalar ----
        with (
            tc.tile_pool(name="fin", bufs=1) as fin,
            tc.tile_pool(name="fpsum", bufs=1, space=bass.MemorySpace.PSUM) as fpsum,
        ):
            sums = fin.tile([128, 2], F32)
            nc.vector.tensor_reduce(
                sums[:, 0:1], rowmins[:], axis=mybir.AxisListType.X, op=ADD
            )
            nc.vector.tensor_reduce(
                sums[:, 1:2], colmins[:], axis=mybir.AxisListType.X, op=ADD
            )
            tot = fin.tile([128, 1], F32)
            nc.vector.tensor_tensor(tot[:], sums[:, 0:1], sums[:, 1:2], op=ADD)
            ps = fpsum.tile([1, 1], F32)
            nc.tensor.matmul(ps[:], tot[:], ones128[:])
            res = fin.tile([1, 1], F32)
            nc.scalar.mul(res[:], ps[:], 1.0 / float(N))
            nc.sync.dma_start(out_d.ap(), res[:])


_NC_CACHE = {}


def _get_nc(reps=1):
    if reps not in _NC_CACHE:
        nc = bacc.Bacc("TRN2", target_bir_lowering=False, debug=False)
        src_d = nc.dram_tensor("src", [N, D], F32, kind="ExternalInput")
        tgt_d = nc.dram_tensor("tgt", [M, D], F32, kind="ExternalInput")
        out_d = nc.dram_tensor("out", [1, 1], F32, kind="ExternalOutput")
        _build_kernel(nc, src_d, tgt_d, out_d, reps=reps)
        nc.compile()
        _NC_CACHE[reps] = nc
    return _NC_CACHE[reps]


def kernel(source_points: np.ndarray, target_points: np.ndarray) -> np.ndarray:
    src = np.ascontiguousarray(np.asarray(source_points), dtype=np.float32)
    tgt = np.ascontiguousarray(np.asarray(target_points), dtype=np.float32)
    assert src.shape == (B, N, D) and tgt.shape == (B, M, D)

    nc = _get_nc()
    in_maps = [{"src": src[b], "tgt": tgt[b]} for b in range(B)]
    res = run_bass_kernel_spmd(nc, in_maps, list(range(B)))
    return np.stack(
        [res.results[b]["out"].reshape(()) for b in range(B)]
    ).astype(np.float32)


if __name__ == "__main__":
    rng = np.random.default_rng(0)
    s = rng.standard_normal((B, N, D), dtype=np.float32)
    t = rng.standard_normal((B, M, D), dtype=np.float32)
    print(kernel(s, t))



# revision 13
# speedup vs baseline: 3.4135x; 1.1116x over previous
"""Chamfer distance loss kernel for Trainium2 (Bass/Tile), 8-core SPMD.

Problem: B=8 batches of N=8192 source / M=8192 target 3-D points.
  dist[n,m] = |s_n|^2 + |t_m|^2 - 2 s.t
  chamfer[b] = mean_n min_m dist + mean_m min_n dist

Sharding: data-parallel over batch; core b handles batch b end-to-end and
emits one scalar. No cross-core communication.

Per-core pipeline:
  PE  : K=5 augmented matmul [s,1,|s|^2].[-2t,|t|^2,1] -> PSUM fp32 = dist
  ACT : PSUM -> SBUF bf16 cast (plain Copy)
  DVE : bf16 2x tensor_tensor min-accumulate (col) + binary fold tree (row)
  PE  : transpose col accumulator for the cross-partition min; ones-matmul
        for the final partition sum
"""

import ml_dtypes
import numpy as np

import concourse.bacc as bacc
import concourse.bass as bass
import concourse.mybir as mybir
import concourse.tile as tile
from concourse.bass_utils import run_bass_kernel_spmd

B = 8
N = 8192  # source points per batch
M = 8192  # target points per batch
D = 3

NT = N // 128  # 64 source tiles of 128
QCH = 2048     # ACT/DVE chunk width (4 PSUM banks)
NH = M // QCH  # 4 chunks per source tile row
BIG = 60000.0  # > any squared distance here, fp16-safe

F32 = mybir.dt.float32
F32R = mybir.dt.float32r  # fp32 bits, full-rate PE streaming (1 cyc/col at N>=256)
F16 = mybir.dt.bfloat16
MIN = mybir.AluOpType.min
ADD = mybir.AluOpType.add


def _build_kernel(nc: bass.Bass, src_d, tgt_d, out_d, reps=1):
    tc_ctx = tile.TileContext(nc)
    with tc_ctx as tc, tc.tile_pool(name="const", bufs=1) as cpool:
        with tc.tile_pool(name="prep", bufs=1) as prep:
            # Persistent SBUF tensors.
            # K=7 augmented operands (fp32r): the cross term rides rows 0-2
            # with coords pre-rounded to fp32r (exact products of perturbed
            # points), and each squared-norm rides as a hi/lo fp32r pair so
            # PSUM receives the full distance at ~fp32 precision while the
            # PE streams at full rate (1 cyc/col).
            #   aug_s rows: s_x, s_y, s_z, 1, 1, |s|^2_hi, |s|^2_lo
            #   aug_t rows: -2t_x, -2t_y, -2t_z, |t|^2_hi, |t|^2_lo, 1, 1
            aug_s_r = cpool.tile([7, N], F32R)
            aug_t_r = cpool.tile([7, M], F32R)
            col_acc = cpool.tile([128, M], F16)   # max over even tiles of -dist
            col_acc2 = cpool.tile([128, M], F16)  # max over odd tiles of -dist
            rowmins = cpool.tile([128, NT], F32)  # min over m of dist, [p, c]
            colmins = cpool.tile([128, NT], F32)  # per-128-m-chunk col mins
            ident = cpool.tile([128, 128], F16)   # identity for PE transpose
            ones128 = cpool.tile([128, 1], F32)   # final partition-sum weights

            id_dram = nc.inline_tensor(np.eye(128, dtype=np.float32).astype(ml_dtypes.bfloat16), name="ident")
            nc.sync.dma_start(ident[:], id_dram.ap())
            nc.gpsimd.memset(ones128[:], 1.0)

            # ---- input prep ----
            # DVE/ACT ops can only address partition bases {0,32,64,96}, so
            # each aug row group is produced (with fp32r rounding) in a
            # partition-0-based staging tile and DMA'd into place; DMA from
            # an fp32r source keeps the rounded provenance the fp32r matmul
            # verifier demands.  Norms are computed FROM the rounded coords,
            # so PSUM receives the exact squared distance of the perturbed
            # point set (plus the tiny hi/lo residual).
            stage = prep.tile([3, M], F32, tag="stage")   # raw coords / scratch
            crd_r = prep.tile([3, M], F32R, tag="crdr")   # rounded coords / scratch
            sq = prep.tile([3, M], F32, tag="sq")
            w_t = prep.tile([3, 1], F32)
            w_s = prep.tile([3, 1], F32)
            nc.gpsimd.memset(w_t[:], 0.25)
            nc.gpsimd.memset(w_s[:], 1.0)
            SUB = mybir.AluOpType.subtract

            def _prep_side(src_dram, n_elems, aug, coord_scale, w, hi_row,
                           ones_row, tag, sign=1.0):
                # sign=-1 negates this side's rows so PSUM gets -dist
                # (all reductions then become max, which pool supports).
                nc.sync.dma_start(
                    stage[:, 0:n_elems], src_dram.ap().rearrange("n d -> d n")
                )
                # rounded (scaled) coords -> aug rows 0-2
                if coord_scale == 1.0:
                    nc.vector.tensor_copy(crd_r[:, 0:n_elems], stage[:, 0:n_elems])
                else:
                    nc.vector.tensor_scalar_mul(
                        crd_r[:, 0:n_elems], stage[:, 0:n_elems], coord_scale
                    )
                nc.sync.dma_start(aug[0:3, :], crd_r[:, 0:n_elems])
                # norm^2 = w * sum of squares of the (scaled) rounded coords
                nc.scalar.square(sq[:, 0:n_elems], crd_r[:, 0:n_elems].bitcast(F32))
                nsq = stage[0:1]  # raw coords dead once crd_r is built
                with tc.tile_pool(
                    name="psum_prep" + tag, bufs=2, space=bass.MemorySpace.PSUM
                ) as pprep:
                    for quarter in range(n_elems // 2048):
                        pt = pprep.tile([1, 2048], F32)
                        for q in range(4):
                            mq = quarter * 2048 + q * 512
                            nc.tensor.matmul(
                                pt[:, q * 512:(q + 1) * 512],
                                w[:],
                                sq[:, mq:mq + 512],
                            )
                        nc.scalar.mul(
                            nsq[:, quarter * 2048:(quarter + 1) * 2048], pt[:],
                            sign,
                        )
                # hi/lo split on the fp32r lattice, staged through crd_r[0:1]
                nc.vector.tensor_copy(crd_r[0:1, 0:n_elems], nsq[:, 0:n_elems])
                nc.sync.dma_start(aug[hi_row:hi_row + 1, :], crd_r[0:1, 0:n_elems])
                nc.vector.tensor_tensor(
                    crd_r[0:1, 0:n_elems], nsq[:, 0:n_elems],
                    crd_r[0:1, 0:n_elems].bitcast(F32), op=SUB,
                )
                nc.sync.dma_start(
                    aug[hi_row + 1:hi_row + 2, :], crd_r[0:1, 0:n_elems]
                )
                # ones rows (sign-carrying)
                nc.gpsimd.memset(stage[0:2, 0:n_elems], sign)
                nc.vector.tensor_copy(crd_r[0:2, 0:n_elems], stage[0:2, 0:n_elems])
                nc.sync.dma_start(
                    aug[ones_row:ones_row + 2, :], crd_r[0:2, 0:n_elems]
                )

            _prep_side(tgt_d, M, aug_t_r, -2.0, w_t, 3, 5, "t")
            _prep_side(src_d, N, aug_s_r, -1.0, w_s, 5, 3, "s", sign=-1.0)

        # ---- main loop (reps>1 only for exec-time measurement) ----
        # PSUM/d16 hold NEGATED distances; all reductions are max.
        MAX = mybir.AluOpType.max
        for _rep in range(reps):
          with (
            tc.tile_pool(name="dpsum", bufs=2, space=bass.MemorySpace.PSUM) as dpsum,
            tc.tile_pool(name="d16", bufs=3) as d16p,
            tc.tile_pool(name="rowacc", bufs=2) as rowp,
          ):
            for c in range(NT):
                lhsT = aug_s_r[:, c * 128:(c + 1) * 128]
                d16 = d16p.tile([128, M], F16)
                for h in range(NH):
                    dps = dpsum.tile([128, QCH], F32)
                    for q in range(QCH // 512):
                        mq = h * QCH + q * 512
                        nc.tensor.matmul(
                            dps[:, q * 512:(q + 1) * 512],
                            lhsT,
                            aug_t_r[:, mq:mq + 512],
                        )
                    # fp32 PSUM -> fp16 SBUF slice of the full row block
                    nc.scalar.copy(d16[:, h * QCH:(h + 1) * QCH], dps[:])
                # column (max over n of -dist): two independent accumulator
                # chains so successive DVE ops aren't serialized on one
                # dependency chain
                acc = col_acc if c % 2 == 0 else col_acc2
                if c < 2:
                    nc.vector.tensor_copy(acc[:], d16[:])
                else:
                    nc.vector.tensor_tensor(acc[:], d16[:], acc[:], op=MAX)
                # row (max over m of -dist): binary fold tree + short reduce
                rowh = rowp.tile([128, M // 2], F16)
                nc.vector.tensor_tensor(
                    rowh[:], d16[:, 0:M // 2], d16[:, M // 2:M], op=MAX
                )
                for w in (M // 4, M // 8, M // 16):
                    nc.vector.tensor_tensor(
                        rowh[:, 0:w], rowh[:, 0:w], rowh[:, w:2 * w], op=MAX
                    )
                nc.vector.tensor_reduce(
                    rowmins[:, c:c + 1], rowh[:, 0:M // 16],
                    axis=mybir.AxisListType.X, op=MAX,
                )

        # ---- merge the two column chains ----
        nc.vector.tensor_tensor(col_acc[:], col_acc2[:], col_acc[:], op=MAX)

        # ---- column partition-reduce via PE transpose ----
        with tc.tile_pool(name="tpsum", bufs=4, space=bass.MemorySpace.PSUM) as tpsum:
            for c in range(NT):
                tps = tpsum.tile([128, 128], F16)
                nc.tensor.transpose(tps[:], col_acc[:, c * 128:(c + 1) * 128], ident[:])
                nc.vector.tensor_reduce(
                    colmins[:, c:c + 1], tps[:], axis=mybir.AxisListType.X, op=MAX
                )

        # ---- final scalar ----
        with (
            tc.tile_pool(name="fin", bufs=1) as fin,
            tc.tile_pool(name="fpsum", bufs=1, space=bass.MemorySpace.PSUM) as fpsum,
        ):
            sums = fin.tile([128, 2], F32)
            nc.vector.tensor_reduce(
                sums[:, 0:1], rowmins[:], axis=mybir.AxisListType.X, op=ADD
            )
            nc.vector.tensor_reduce(
                sums[:, 1:2], colmins[:], axis=mybir.AxisListType.X, op=ADD
            )
            tot = fin.tile([128, 1], F32)
            nc.vector.tensor_tensor(tot[:], sums[:, 0:1], sums[:, 1:2], op=ADD)
            ps = fpsum.tile([1, 1], F32)
            nc.tensor.matmul(ps[:], tot[:], ones128[:])
            res = fin.tile([1, 1], F32)
            nc.scalar.mul(res[:], ps[:], -1.0 / float(N))
            nc.sync.dma_start(out_d.ap(), res[:])


_NC_CACHE = {}


def _get_nc(reps=1):
    if reps not in _NC_CACHE:
        nc = bacc.Bacc("TRN2", target_bir_lowering=False, debug=False)
        src_d = nc.dram_tensor("src", [N, D], F32, kind="ExternalInput")
        tgt_d = nc.dram_tensor("tgt", [M, D], F32, kind="ExternalInput")
        out_d = nc.dram_tensor("out", [1, 1], F32, kind="ExternalOutput")
        _build_kernel(nc, src_d, tgt_d, out_d, reps=reps)
        nc.compile()
        _NC_CACHE[reps] = nc
    return _NC_CACHE[reps]


def kernel(source_points: np.ndarray, target_points: np.ndarray) -> np.ndarray:
    src = np.ascontiguousarray(np.asarray(source_points), dtype=np.float32)
    tgt = np.ascontiguousarray(np.asarray(target_points), dtype=np.float32)
    assert src.shape == (B, N, D) and tgt.shape == (B, M, D)

    nc = _get_nc()
    in_maps = [{"src": src[b], "tgt": tgt[b]} for b in range(B)]
    res = run_bass_kernel_spmd(nc, in_maps, list(range(B)))
    return np.stack(
        [res.results[b]["out"].reshape(()) for b in range(B)]
    ).astype(np.float32)


if __name__ == "__main__":
    rng = np.random.default_rng(0)
    s = rng.standard_normal((B, N, D), dtype=np.float32)
    t = rng.standard_normal((B, M, D), dtype=np.float32)
    print(kernel(s, t))

